# revision 2
# baseline (speedup 1.0000x reference)
"""AttentionWide (t=2048, e=512, h=8) on 8 TRN2 NeuronCores.

Tensor-parallel over heads: core i owns head i (columns i*512:(i+1)*512 of
Wk/Wq/Wv, rows i*512:(i+1)*512 of Wu).  Each core computes its head's
attention and the partial unifyheads product; a ReduceScatter sums the
partials across cores, each core returning a row-shard of the final output.

All big matmuls run in bf16 with fp32 PSUM accumulation.  Layout trick:
everything is computed in "transposed" [e, t] / [tk, tq] layouts so that no
on-device transposes are needed (x/y are transposed once on the host):
    kT = Wk^T @ xT          lhsT=Wk (natural),  rhs=xT
    qT = Wq^T @ yT          lhsT=Wq,            rhs=yT
    vT = Wv^T @ xT          lhsT=Wv,            rhs=xT
    vW = vT^T @ Wu          lhsT=vT slices,     rhs=Wu      ([t, e] natural)
    scoresT[tk,tq] = kT^T@qT    lhsT=kT slices, rhs=qT
    expT = exp(scoresT * e^-0.5)   (softmax w/o max-subtraction; scores ~ +-1)
    out[tq,:] = (expT^T @ vW) / (expT^T @ ones)   lhsT=expT slices (shared)
"""

import os
import numpy as np
import ml_dtypes

T, E, H = 2048, 512, 8
NCORES = 8
TB = 512          # matmul moving-operand block (free dim; one fp32 PSUM bank)
NE = E // 128     # 4  partition tiles of the emb dim
NT = T // 128     # 16 partition tiles of the seq dim
NB = T // TB      # 4  seq blocks
RS_ROWS = TB // NCORES  # 64 rows per core per block from ReduceScatter

_cache = {}
last_result = None


def _build_nc():
    from concourse import bacc, tile
    from concourse.bass import mybir

    bf16 = mybir.dt.bfloat16
    f32 = mybir.dt.float32

    nc = bacc.Bacc(
        "TRN2", target_bir_lowering=False, debug=False, num_devices=NCORES
    )

    xT = nc.dram_tensor("xT", [E, T], bf16, kind="ExternalInput")
    yT = nc.dram_tensor("yT", [E, T], bf16, kind="ExternalInput")
    wk = nc.dram_tensor("wk", [E, E], bf16, kind="ExternalInput")
    wq = nc.dram_tensor("wq", [E, E], bf16, kind="ExternalInput")
    wv = nc.dram_tensor("wv", [E, E], bf16, kind="ExternalInput")
    wu = nc.dram_tensor("wu", [E, E], bf16, kind="ExternalInput")
    out_ext = nc.dram_tensor("out", [NB, RS_ROWS, E], f32, kind="ExternalOutput")

    with tile.TileContext(nc) as tc:
        with (
            tc.tile_pool(name="persist", bufs=1) as persist,
            tc.tile_pool(name="work", bufs=4) as work,
            tc.tile_pool(name="expp", bufs=32) as expp,
            tc.tile_pool(name="psum", bufs=2, space="PSUM") as psum_pool,
            tc.tile_pool(name="dram", bufs=1, space="DRAM") as dram,
        ):
            def load_rows(prefix, src, n):
                tiles = []
                for j in range(NE):
                    t = persist.tile(
                        [128, n], bf16, tag=f"{prefix}{j}", name=f"{prefix}{j}"
                    )
                    nc.sync.dma_start(t[:], src[j * 128 : (j + 1) * 128, :])
                    tiles.append(t)
                return tiles

            xT_sb = load_rows("xTs", xT, T)
            yT_sb = load_rows("yTs", yT, T)
            wk_sb = load_rows("wks", wk, E)
            wq_sb = load_rows("wqs", wq, E)
            wv_sb = load_rows("wvs", wv, E)
            wu_sb = load_rows("wus", wu, E)

            kT_sb = [
                persist.tile([128, T], bf16, tag=f"kTs{m}", name=f"kTs{m}")
                for m in range(NE)
            ]
            qT_sb = [
                persist.tile([128, T], bf16, tag=f"qTs{m}", name=f"qTs{m}")
                for m in range(NE)
            ]
            vT_sb = [
                persist.tile([128, T], bf16, tag=f"vTs{m}", name=f"vTs{m}")
                for m in range(NE)
            ]
            vW_sb = [
                persist.tile([128, E], bf16, tag=f"vWs{t}", name=f"vWs{t}")
                for t in range(NT)
            ]

            ones_sb = persist.tile([128, 1], bf16, tag="ones", name="ones")
            nc.vector.memset(ones_sb[:], 1.0)
            zbias = persist.tile([128, 1], f32, tag="zbias", name="zbias")
            nc.vector.memset(zbias[:], 0.0)

            def project(dst, w, src):
                # dst[m][:, tb] = sum_j w[j][:, m-slice].T @ src[j][:, tb]
                for m in range(NE):
                    for tb in range(NB):
                        ps = psum_pool.tile(
                            [128, TB], f32, tag="mm", bufs=4, name="ps_proj"
                        )
                        for j in range(NE):
                            nc.tensor.matmul(
                                ps[:],
                                w[j][:, m * 128 : (m + 1) * 128],
                                src[j][:, tb * TB : (tb + 1) * TB],
                                start=(j == 0),
                                stop=(j == NE - 1),
                            )
                        nc.vector.tensor_copy(
                            dst[m][:, tb * TB : (tb + 1) * TB], ps[:]
                        )

            project(kT_sb, wk_sb, xT_sb)
            project(qT_sb, wq_sb, yT_sb)
            project(vT_sb, wv_sb, xT_sb)

            # vW[t, :] = v @ Wu  (natural [t, e] layout)
            for t in range(NT):
                ps = psum_pool.tile([128, E], f32, tag="mm", bufs=4, name="ps_vw")
                for j in range(NE):
                    nc.tensor.matmul(
                        ps[:],
                        vT_sb[j][:, t * 128 : (t + 1) * 128],
                        wu_sb[j][:],
                        start=(j == 0),
                        stop=(j == NE - 1),
                    )
                nc.vector.tensor_copy(vW_sb[t][:], ps[:])

            SCALE = float(E) ** -0.5
            parts = [
                dram.tile([TB, E], f32, tag=f"part{b}", name=f"part{b}")
                for b in range(NB)
            ]
            rs_outs = [
                dram.tile([RS_ROWS, E], f32, tag=f"rso{b}", name=f"rso{b}")
                for b in range(NB)
            ]

            for b in range(NB):
                # scoresT[tk, tq-block b] then exp
                exp_tiles = []
                for tk in range(NT):
                    ps = psum_pool.tile(
                        [128, TB], f32, tag="mm", bufs=4, name="ps_sc"
                    )
                    for m in range(NE):
                        nc.tensor.matmul(
                            ps[:],
                            kT_sb[m][:, tk * 128 : (tk + 1) * 128],
                            qT_sb[m][:, b * TB : (b + 1) * TB],
                            start=(m == 0),
                            stop=(m == NE - 1),
                        )
                    et = expp.tile([128, TB], bf16, tag="expT", bufs=32, name="et")
                    nc.scalar.activation(
                        et[:],
                        ps[:],
                        mybir.ActivationFunctionType.Exp,
                        bias=zbias[:],
                        scale=SCALE,
                    )
                    exp_tiles.append(et)

                # out rows for this block: accumulate over tk, then normalize
                for qi in range(TB // 128):
                    po = psum_pool.tile([128, E], f32, tag="acc", bufs=2, name="po")
                    pv = psum_pool.tile([128, 1], f32, tag="sums", bufs=2, name="pv")
                    for tk in range(NT):
                        lhs = exp_tiles[tk][:, qi * 128 : (qi + 1) * 128]
                        nc.tensor.matmul(
                            po[:],
                            lhs,
                            vW_sb[tk][:],
                            start=(tk == 0),
                            stop=(tk == NT - 1),
                        )
                        nc.tensor.matmul(
                            pv[:],
                            lhs,
                            ones_sb[:],
                            start=(tk == 0),
                            stop=(tk == NT - 1),
                        )
                    rec = work.tile([128, 1], f32, tag="rec", bufs=4, name="rec")
                    nc.vector.reciprocal(rec[:], pv[:])
                    ot = work.tile([128, E], f32, tag="ot", bufs=4, name="ot")
                    nc.vector.tensor_scalar_mul(ot[:], po[:], rec[:])
                    nc.sync.dma_start(
                        parts[b][qi * 128 : (qi + 1) * 128, :], ot[:]
                    )

                nc.gpsimd.collective_compute(
                    "ReduceScatter",
                    mybir.AluOpType.add,
                    replica_groups=[list(range(NCORES))],
                    ins=[parts[b][:]],
                    outs=[rs_outs[b][:]],
                )
                nc.sync.dma_start(out_ext[b], rs_outs[b][:])

    nc.compile()
    return nc


def kernel(x, y, Wk, Wq, Wv, Wu, bu):
    global last_result
    from concourse.bass_utils import run_bass_kernel_spmd

    if "nc" not in _cache:
        _cache["nc"] = _build_nc()
    nc = _cache["nc"]

    bf = ml_dtypes.bfloat16
    xT = np.ascontiguousarray(np.asarray(x, np.float32).T).astype(bf)
    yT = np.ascontiguousarray(np.asarray(y, np.float32).T).astype(bf)
    Wk = np.asarray(Wk, np.float32)
    Wq = np.asarray(Wq, np.float32)
    Wv = np.asarray(Wv, np.float32)
    Wu = np.asarray(Wu, np.float32)

    in_maps = []
    for i in range(NCORES):
        sl = slice(i * E, (i + 1) * E)
        in_maps.append(
            {
                "xT": xT,
                "yT": yT,
                "wk": np.ascontiguousarray(Wk[:, sl]).astype(bf),
                "wq": np.ascontiguousarray(Wq[:, sl]).astype(bf),
                "wv": np.ascontiguousarray(Wv[:, sl]).astype(bf),
                "wu": np.ascontiguousarray(Wu[sl, :]).astype(bf),
            }
        )

    trace = os.environ.get("KERNEL_TRACE", "0") == "1"
    res = run_bass_kernel_spmd(
        nc, in_maps, core_ids=list(range(NCORES)), trace=trace
    )
    last_result = res

    out_full = np.empty((T, E), np.float32)
    for i in range(NCORES):
        o = np.asarray(res.results[i]["out"], np.float32)  # [NB, RS_ROWS, E]
        for b in range(NB):
            r0 = b * TB + i * RS_ROWS
            out_full[r0 : r0 + RS_ROWS] = o[b]
    out_full = out_full + np.asarray(bu, np.float32)[None, :]
    return out_full[None]


# revision 3
# speedup vs baseline: 1.1656x; 1.1656x over previous
"""AttentionWide (t=2048, e=512, h=8) on 8 TRN2 NeuronCores.

Tensor-parallel over heads: core i owns head i (columns i*512:(i+1)*512 of
Wk/Wq/Wv, rows i*512:(i+1)*512 of Wu).  Each core computes its head's
attention and the partial unifyheads product; chunked ReduceScatters sum the
partials across cores, each core returning row-shards of the final output.

Weight folding (host-side, exact algebra — the head dim equals emb here so
no information is lost):
    scores = q k^T = (y Wq)(x Wk)^T = y (Wq Wk^T) x^T
      ->  G  = Wk Wq^T   [e, e]   (host)
          g  = x G                 (device)
          scoresT[tk, tq] = g y^T  (device)
    out = attn @ v @ Wu = attn @ (x Wv Wu)
      ->  W2 = Wv Wu     [e, e]   (host)
          vW = x W2                (device)
This removes two of the four projection matmul groups.

Device compute in bf16 with fp32 PSUM accumulation, in "transposed"
layouts so no on-device transposes are needed (x/y transposed on host):
    gT = G^T @ xT            lhsT=G (natural),  rhs=xT       [e, t]
    vW = xT^T @ W2           lhsT=xT slices,    rhs=W2       [t, e]
    scoresT[tk,tq] = gT^T yT lhsT=gT slices,    rhs=yT
    expT = exp(scoresT * e^-0.5)  (softmax w/o max-subtraction; |scores|<~2)
    out[tq,:] = (expT^T @ vW) / (expT^T @ ones)  lhsT=expT slices (shared)
"""

import os
import numpy as np
import ml_dtypes

T, E, H = 2048, 512, 8
NCORES = 8
TB = 512          # matmul moving-operand block (free dim; one fp32 PSUM bank)
NE = E // 128     # 4  partition tiles of the emb dim
NT = T // 128     # 16 partition tiles of the seq dim
NB = T // TB      # 4  seq blocks
NCH = 8           # ReduceScatter chunks (256 rows each)
CH_ROWS = T // NCH            # 256
RS_ROWS = CH_ROWS // NCORES   # 32 rows per core per chunk

_cache = {}
last_result = None


def _build_nc():
    from concourse import bacc, tile
    from concourse.bass import mybir

    bf16 = mybir.dt.bfloat16
    f32 = mybir.dt.float32

    nc = bacc.Bacc(
        "TRN2", target_bir_lowering=False, debug=False, num_devices=NCORES
    )

    xT = nc.dram_tensor("xT", [E, T], bf16, kind="ExternalInput")
    yT = nc.dram_tensor("yT", [E, T], bf16, kind="ExternalInput")
    gw = nc.dram_tensor("gw", [E, E], bf16, kind="ExternalInput")   # Wk Wq^T
    w2 = nc.dram_tensor("w2", [E, E], bf16, kind="ExternalInput")   # Wv Wu
    out_ext = nc.dram_tensor("out", [NCH, RS_ROWS, E], f32, kind="ExternalOutput")

    with tile.TileContext(nc) as tc:
        with (
            tc.tile_pool(name="persist", bufs=1) as persist,
            tc.tile_pool(name="work", bufs=4) as work,
            tc.tile_pool(name="expp", bufs=32) as expp,
            tc.tile_pool(name="psum", bufs=2, space="PSUM") as psum_pool,
            tc.tile_pool(name="dram", bufs=1, space="DRAM") as dram,
        ):
            def alloc_rows(prefix, n):
                return [
                    persist.tile(
                        [128, n], bf16, tag=f"{prefix}{j}", name=f"{prefix}{j}"
                    )
                    for j in range(NE)
                ]

            xT_sb = alloc_rows("xTs", T)
            yT_sb = alloc_rows("yTs", T)
            gw_sb = alloc_rows("gws", E)
            w2_sb = alloc_rows("w2s", E)

            # DMA order = need order: gw, xT col-chunk 0, w2, rest of xT,
            # then yT per column block.  Column-chunked so the first
            # projection matmuls can start after ~1MB has landed.
            for j in range(NE):
                nc.sync.dma_start(gw_sb[j][:], gw[j * 128 : (j + 1) * 128, :])
            for j in range(NE):
                nc.sync.dma_start(
                    xT_sb[j][:, 0:TB], xT[j * 128 : (j + 1) * 128, 0:TB]
                )
            for j in range(NE):
                nc.sync.dma_start(w2_sb[j][:], w2[j * 128 : (j + 1) * 128, :])
            for tb in range(1, NB):
                for j in range(NE):
                    nc.sync.dma_start(
                        xT_sb[j][:, tb * TB : (tb + 1) * TB],
                        xT[j * 128 : (j + 1) * 128, tb * TB : (tb + 1) * TB],
                    )
            for tb in range(NB):
                for j in range(NE):
                    nc.sync.dma_start(
                        yT_sb[j][:, tb * TB : (tb + 1) * TB],
                        yT[j * 128 : (j + 1) * 128, tb * TB : (tb + 1) * TB],
                    )

            gT_sb = alloc_rows("gTs", T)
            vW_sb = [
                persist.tile([128, E], bf16, tag=f"vWs{t}", name=f"vWs{t}")
                for t in range(NT)
            ]

            ones_sb = persist.tile([128, 1], bf16, tag="ones", name="ones")
            nc.vector.memset(ones_sb[:], 1.0)
            zbias = persist.tile([128, 1], f32, tag="zbias", name="zbias")
            nc.vector.memset(zbias[:], 0.0)

            # gT[m][:, tk] = sum_j G[j][:, m-slice].T @ xT[j][:, tk-block]
            for m in range(NE):
                for tb in range(NB):
                    ps = psum_pool.tile(
                        [128, TB], f32, tag="mm", bufs=4, name="ps_g"
                    )
                    for j in range(NE):
                        nc.tensor.matmul(
                            ps[:],
                            gw_sb[j][:, m * 128 : (m + 1) * 128],
                            xT_sb[j][:, tb * TB : (tb + 1) * TB],
                            start=(j == 0),
                            stop=(j == NE - 1),
                        )
                    nc.vector.tensor_copy(gT_sb[m][:, tb * TB : (tb + 1) * TB], ps[:])

            # vW[t, :] = x @ W2   (natural [t, e] layout)
            for t in range(NT):
                ps = psum_pool.tile([128, E], f32, tag="mm", bufs=4, name="ps_vw")
                for j in range(NE):
                    nc.tensor.matmul(
                        ps[:],
                        xT_sb[j][:, t * 128 : (t + 1) * 128],
                        w2_sb[j][:],
                        start=(j == 0),
                        stop=(j == NE - 1),
                    )
                nc.vector.tensor_copy(vW_sb[t][:], ps[:])

            SCALE = float(E) ** -0.5
            parts = [
                dram.tile([CH_ROWS, E], f32, tag=f"part{c}", name=f"part{c}")
                for c in range(NCH)
            ]
            rs_outs = [
                dram.tile([RS_ROWS, E], f32, tag=f"rso{c}", name=f"rso{c}")
                for c in range(NCH)
            ]

            for b in range(NB):
                # scoresT[tk, tq-block b] then exp
                exp_tiles = []
                for tk in range(NT):
                    ps = psum_pool.tile(
                        [128, TB], f32, tag="mm", bufs=4, name="ps_sc"
                    )
                    for m in range(NE):
                        nc.tensor.matmul(
                            ps[:],
                            gT_sb[m][:, tk * 128 : (tk + 1) * 128],
                            yT_sb[m][:, b * TB : (b + 1) * TB],
                            start=(m == 0),
                            stop=(m == NE - 1),
                        )
                    et = expp.tile([128, TB], bf16, tag="expT", bufs=32, name="et")
                    nc.scalar.activation(
                        et[:],
                        ps[:],
                        mybir.ActivationFunctionType.Exp,
                        bias=zbias[:],
                        scale=SCALE,
                    )
                    exp_tiles.append(et)

                # out rows for this block: accumulate over tk, then normalize
                for qi in range(TB // 128):
                    po = psum_pool.tile([128, E], f32, tag="acc", bufs=2, name="po")
                    pv = psum_pool.tile([128, 1], f32, tag="sums", bufs=2, name="pv")
                    for tk in range(NT):
                        lhs = exp_tiles[tk][:, qi * 128 : (qi + 1) * 128]
                        nc.tensor.matmul(
                            po[:],
                            lhs,
                            vW_sb[tk][:],
                            start=(tk == 0),
                            stop=(tk == NT - 1),
                        )
                        nc.tensor.matmul(
                            pv[:],
                            lhs,
                            ones_sb[:],
                            start=(tk == 0),
                            stop=(tk == NT - 1),
                        )
                    rec = work.tile([128, 1], f32, tag="rec", bufs=4, name="rec")
                    nc.vector.reciprocal(rec[:], pv[:])
                    ot = work.tile([128, E], f32, tag="ot", bufs=4, name="ot")
                    nc.vector.tensor_scalar_mul(ot[:], po[:], rec[:])
                    ch = 2 * b + qi // 2
                    nc.sync.dma_start(
                        parts[ch][(qi % 2) * 128 : (qi % 2 + 1) * 128, :], ot[:]
                    )
                    if qi % 2 == 1:
                        nc.gpsimd.collective_compute(
                            "ReduceScatter",
                            mybir.AluOpType.add,
                            replica_groups=[list(range(NCORES))],
                            ins=[parts[ch][:]],
                            outs=[rs_outs[ch][:]],
                        )
                        nc.sync.dma_start(out_ext[ch], rs_outs[ch][:])

    nc.compile()
    return nc


def kernel(x, y, Wk, Wq, Wv, Wu, bu):
    global last_result
    from concourse.bass_utils import run_bass_kernel_spmd

    if "nc" not in _cache:
        _cache["nc"] = _build_nc()
    nc = _cache["nc"]

    bf = ml_dtypes.bfloat16
    x = np.asarray(x, np.float32)
    y = np.asarray(y, np.float32)
    Wk = np.asarray(Wk, np.float32)
    Wq = np.asarray(Wq, np.float32)
    Wv = np.asarray(Wv, np.float32)
    Wu = np.asarray(Wu, np.float32)

    xT = np.ascontiguousarray(x.T).astype(bf)
    yT = np.ascontiguousarray(y.T).astype(bf)

    in_maps = []
    for i in range(NCORES):
        sl = slice(i * E, (i + 1) * E)
        G = Wk[:, sl] @ Wq[:, sl].T        # [e, e] fp32 on host
        W2 = Wv[:, sl] @ Wu[sl, :]         # [e, e] fp32 on host
        in_maps.append(
            {
                "xT": xT,
                "yT": yT,
                "gw": G.astype(bf),
                "w2": W2.astype(bf),
            }
        )

    trace = os.environ.get("KERNEL_TRACE", "0") == "1"
    res = run_bass_kernel_spmd(
        nc, in_maps, core_ids=list(range(NCORES)), trace=trace
    )
    last_result = res

    out_full = np.empty((T, E), np.float32)
    for i in range(NCORES):
        o = np.asarray(res.results[i]["out"], np.float32)  # [NCH, RS_ROWS, E]
        for c in range(NCH):
            r0 = c * CH_ROWS + i * RS_ROWS
            out_full[r0 : r0 + RS_ROWS] = o[c]
    out_full = out_full + np.asarray(bu, np.float32)[None, :]
    return out_full[None]


# revision 8
# speedup vs baseline: 1.1743x; 1.0075x over previous
"""AttentionWide (t=2048, e=512, h=8) on 8 TRN2 NeuronCores.

Tensor-parallel over heads: core i owns head i (columns i*512:(i+1)*512 of
Wk/Wq/Wv, rows i*512:(i+1)*512 of Wu).  Each core computes its head's
attention and the partial unifyheads product; chunked ReduceScatters sum the
partials across cores, each core returning row-shards of the final output.

Weight folding (host-side, exact algebra — the head dim equals emb here so
no information is lost):
    scores = q k^T = (y Wq)(x Wk)^T = y (Wq Wk^T) x^T
      ->  G  = Wk Wq^T   [e, e]   (host)
          g  = x G                 (device)
          scoresT[tk, tq] = g y^T  (device)
    out = attn @ v @ Wu = attn @ (x Wv Wu)
      ->  W2 = Wv Wu     [e, e]   (host)
          vW = x W2                (device)
This removes two of the four projection matmul groups.

Device compute in bf16 with fp32 PSUM accumulation, in "transposed"
layouts so no on-device transposes are needed (x/y transposed on host):
    gT = G^T @ xT            lhsT=G (natural),  rhs=xT       [e, t]
    vW = xT^T @ W2           lhsT=xT slices,    rhs=W2       [t, e]
    scoresT[tk,tq] = gT^T yT lhsT=gT slices,    rhs=yT
    expT = exp(scoresT * e^-0.5)  (softmax w/o max-subtraction; |scores|<~2)
    out[tq,:] = (expT^T @ vW) / (expT^T @ ones)  lhsT=expT slices (shared)
"""

import os
import numpy as np
import ml_dtypes

T, E, H = 2048, 512, 8
NCORES = 8
TB = 512          # matmul moving-operand block (free dim; one fp32 PSUM bank)
NE = E // 128     # 4  partition tiles of the emb dim
NT = T // 128     # 16 partition tiles of the seq dim
NB = T // TB      # 4  seq blocks
NCH = 4           # ReduceScatter chunks (one per block)
CH_ROWS = T // NCH            # 512
RS_ROWS = CH_ROWS // NCORES   # 64 rows per core per chunk
EP = E + 1        # vW columns + folded ones column (softmax denominator)
NA = 257          # first-half free dim of the split final matmul
NBC = EP - NA     # 256; its last column holds the denominator

_cache = {}
last_result = None


def _build_nc():
    from concourse import bacc, tile
    from concourse.bass import mybir

    bf16 = mybir.dt.bfloat16
    f32 = mybir.dt.float32

    nc = bacc.Bacc(
        "TRN2", target_bir_lowering=False, debug=False, num_devices=NCORES
    )

    xT = nc.dram_tensor("xT", [E, T], bf16, kind="ExternalInput")
    yT = nc.dram_tensor("yT", [E, T], bf16, kind="ExternalInput")
    gw = nc.dram_tensor("gw", [E, E], bf16, kind="ExternalInput")   # Wk Wq^T
    w2 = nc.dram_tensor("w2", [E, E], bf16, kind="ExternalInput")   # Wv Wu
    out_ext = nc.dram_tensor("out", [NCH, RS_ROWS, E], f32, kind="ExternalOutput")

    with tile.TileContext(nc) as tc:
        with (
            tc.tile_pool(name="persist", bufs=1) as persist,
            tc.tile_pool(name="work", bufs=4) as work,
            tc.tile_pool(name="expp", bufs=32) as expp,
            tc.tile_pool(name="psum", bufs=2, space="PSUM") as psum_pool,
            tc.tile_pool(name="dram", bufs=1, space="DRAM") as dram,
        ):
            def alloc_rows(prefix, n):
                return [
                    persist.tile(
                        [128, n], bf16, tag=f"{prefix}{j}", name=f"{prefix}{j}"
                    )
                    for j in range(NE)
                ]

            xT_sb = alloc_rows("xTs", T)
            yT_sb = alloc_rows("yTs", T)
            gw_sb = alloc_rows("gws", E)
            w2_sb = alloc_rows("w2s", E)

            # DMA order = need order: gw, xT col-chunk 0, w2, rest of xT,
            # then yT per column block.  Column-chunked so the first
            # projection matmuls can start after ~1MB has landed.
            for j in range(NE):
                nc.sync.dma_start(gw_sb[j][:], gw[j * 128 : (j + 1) * 128, :])
            for tb in range(NB):
                for j in range(NE):
                    nc.sync.dma_start(
                        xT_sb[j][:, tb * TB : (tb + 1) * TB],
                        xT[j * 128 : (j + 1) * 128, tb * TB : (tb + 1) * TB],
                    )
            for j in range(NE):
                nc.sync.dma_start(w2_sb[j][:], w2[j * 128 : (j + 1) * 128, :])
            for tb in range(NB):
                for j in range(NE):
                    nc.sync.dma_start(
                        yT_sb[j][:, tb * TB : (tb + 1) * TB],
                        yT[j * 128 : (j + 1) * 128, tb * TB : (tb + 1) * TB],
                    )

            gT_sb = alloc_rows("gTs", T)
            # vW plus a folded ones column: col E is 1.0, so the final
            # matmul's second half also produces the softmax denominator.
            vW_sb = [
                persist.tile([128, EP], bf16, tag=f"vWs{t}", name=f"vWs{t}")
                for t in range(NT)
            ]

            zbias = persist.tile([128, 1], f32, tag="zbias", name="zbias")
            nc.vector.memset(zbias[:], 0.0)

            # gT[m][:, tk] = sum_j G[j][:, m-slice].T @ xT[j][:, tk-block]
            for m in range(NE):
                for tb in range(NB):
                    ps = psum_pool.tile(
                        [128, TB], f32, tag="mm", bufs=4, name="ps_g"
                    )
                    for j in range(NE):
                        nc.tensor.matmul(
                            ps[:],
                            gw_sb[j][:, m * 128 : (m + 1) * 128],
                            xT_sb[j][:, tb * TB : (tb + 1) * TB],
                            start=(j == 0),
                            stop=(j == NE - 1),
                        )
                    nc.vector.tensor_copy(gT_sb[m][:, tb * TB : (tb + 1) * TB], ps[:])

            # vW[t, :] = x @ W2   (natural [t, e] layout), ones in col E
            for t in range(NT):
                ps = psum_pool.tile([128, E], f32, tag="mm", bufs=4, name="ps_vw")
                for j in range(NE):
                    nc.tensor.matmul(
                        ps[:],
                        xT_sb[j][:, t * 128 : (t + 1) * 128],
                        w2_sb[j][:],
                        start=(j == 0),
                        stop=(j == NE - 1),
                    )
                nc.vector.memset(vW_sb[t][:, E:EP], 1.0)
                nc.vector.tensor_copy(vW_sb[t][:, 0:E], ps[:])

            SCALE = float(E) ** -0.5
            parts = [
                dram.tile([CH_ROWS, E], f32, tag=f"part{c}", name=f"part{c}")
                for c in range(NCH)
            ]
            rs_outs = [
                dram.tile([RS_ROWS, E], f32, tag=f"rso{c}", name=f"rso{c}")
                for c in range(NCH)
            ]

            for b in range(NB):
                # scoresT[tk, tq-block b] then exp
                exp_tiles = []
                for tk in range(NT):
                    ps = psum_pool.tile(
                        [128, TB], f32, tag="mm", bufs=4, name="ps_sc"
                    )
                    for m in range(NE):
                        nc.tensor.matmul(
                            ps[:],
                            gT_sb[m][:, tk * 128 : (tk + 1) * 128],
                            yT_sb[m][:, b * TB : (b + 1) * TB],
                            start=(m == 0),
                            stop=(m == NE - 1),
                        )
                    et = expp.tile([128, TB], bf16, tag="expT", bufs=32, name="et")
                    nc.scalar.activation(
                        et[:],
                        ps[:],
                        mybir.ActivationFunctionType.Exp,
                        bias=zbias[:],
                        scale=SCALE,
                    )
                    exp_tiles.append(et)

                # out rows for this block: accumulate over tk, then normalize.
                # The EP=513 free dim is split 257+256 across two PSUM banks;
                # the last column of pb is the softmax denominator.
                for qi in range(TB // 128):
                    pa = psum_pool.tile([128, NA], f32, tag="acca", bufs=2, name="pa")
                    pb = psum_pool.tile([128, NBC], f32, tag="accb", bufs=2, name="pb")
                    for tk in range(NT):
                        lhs = exp_tiles[tk][:, qi * 128 : (qi + 1) * 128]
                        nc.tensor.matmul(
                            pa[:],
                            lhs,
                            vW_sb[tk][:, 0:NA],
                            start=(tk == 0),
                            stop=(tk == NT - 1),
                        )
                        nc.tensor.matmul(
                            pb[:],
                            lhs,
                            vW_sb[tk][:, NA:EP],
                            start=(tk == 0),
                            stop=(tk == NT - 1),
                        )
                    rec = work.tile([128, 1], f32, tag="rec", bufs=4, name="rec")
                    nc.vector.reciprocal(rec[:], pb[:, NBC - 1 : NBC])
                    ot = work.tile([128, E], f32, tag="ot", bufs=4, name="ot")
                    nc.vector.tensor_scalar_mul(ot[:, 0:NA], pa[:], rec[:])
                    nc.vector.tensor_scalar_mul(
                        ot[:, NA:E], pb[:, 0 : NBC - 1], rec[:]
                    )
                    nc.sync.dma_start(
                        parts[b][qi * 128 : (qi + 1) * 128, :], ot[:]
                    )
                nc.gpsimd.collective_compute(
                    "ReduceScatter",
                    mybir.AluOpType.add,
                    replica_groups=[list(range(NCORES))],
                    ins=[parts[b][:]],
                    outs=[rs_outs[b][:]],
                )
                nc.sync.dma_start(out_ext[b], rs_outs[b][:])

    nc.compile()
    return nc


def kernel(x, y, Wk, Wq, Wv, Wu, bu):
    global last_result
    from concourse.bass_utils import run_bass_kernel_spmd

    if "nc" not in _cache:
        _cache["nc"] = _build_nc()
    nc = _cache["nc"]

    bf = ml_dtypes.bfloat16
    x = np.asarray(x, np.float32)
    y = np.asarray(y, np.float32)
    Wk = np.asarray(Wk, np.float32)
    Wq = np.asarray(Wq, np.float32)
    Wv = np.asarray(Wv, np.float32)
    Wu = np.asarray(Wu, np.float32)

    xT = np.ascontiguousarray(x.T).astype(bf)
    yT = np.ascontiguousarray(y.T).astype(bf)

    in_maps = []
    for i in range(NCORES):
        sl = slice(i * E, (i + 1) * E)
        G = Wk[:, sl] @ Wq[:, sl].T        # [e, e] fp32 on host
        W2 = Wv[:, sl] @ Wu[sl, :]         # [e, e] fp32 on host
        in_maps.append(
            {
                "xT": xT,
                "yT": yT,
                "gw": G.astype(bf),
                "w2": W2.astype(bf),
            }
        )

    trace = os.environ.get("KERNEL_TRACE", "0") == "1"
    res = run_bass_kernel_spmd(
        nc, in_maps, core_ids=list(range(NCORES)), trace=trace
    )
    last_result = res

    out_full = np.empty((T, E), np.float32)
    for i in range(NCORES):
        o = np.asarray(res.results[i]["out"], np.float32)  # [NCH, RS_ROWS, E]
        for c in range(NCH):
            r0 = c * CH_ROWS + i * RS_ROWS
            out_full[r0 : r0 + RS_ROWS] = o[c]
    out_full = out_full + np.asarray(bu, np.float32)[None, :]
    return out_full[None]


# revision 14
# speedup vs baseline: 1.1993x; 1.0213x over previous
"""AttentionWide (t=2048, e=512, h=8) on 8 TRN2 NeuronCores.

Tensor-parallel over heads: core i owns head i (columns i*512:(i+1)*512 of
Wk/Wq/Wv, rows i*512:(i+1)*512 of Wu).  Each core computes its head's
attention and the partial unifyheads product; chunked ReduceScatters sum the
partials across cores, each core returning row-shards of the final output.

Weight folding (host-side, exact algebra — the head dim equals emb here so
no information is lost):
    scores = q k^T = (y Wq)(x Wk)^T = y (Wq Wk^T) x^T
      ->  G  = Wk Wq^T   [e, e]   (host)
          g  = x G                 (device)
          scoresT[tk, tq] = g y^T  (device)
    out = attn @ v @ Wu = attn @ (x Wv Wu)
      ->  W2 = Wv Wu     [e, e]   (host)
          vW = x W2                (device)
This removes two of the four projection matmul groups.

Device compute in bf16 with fp32 PSUM accumulation, in "transposed"
layouts so no on-device transposes are needed (x/y transposed on host):
    gT = G^T @ xT            lhsT=G (natural),  rhs=xT       [e, t]
    vW = xT^T @ W2           lhsT=xT slices,    rhs=W2       [t, e]
    scoresT[tk,tq] = gT^T yT lhsT=gT slices,    rhs=yT
    expT = exp(scoresT * e^-0.5)  (softmax w/o max-subtraction; |scores|<~2)
    out[tq,:] = (expT^T @ vW) / (expT^T @ ones)  lhsT=expT slices (shared)
"""

import os
import numpy as np
import ml_dtypes

T, E, H = 2048, 512, 8
NCORES = 8
TB = 512          # matmul moving-operand block (free dim; one fp32 PSUM bank)
NE = E // 128     # 4  partition tiles of the emb dim
NT = T // 128     # 16 partition tiles of the seq dim
NB = T // TB      # 4  seq blocks
NCH = 4           # ReduceScatter chunks (one per block)
CH_ROWS = T // NCH            # 512
RS_ROWS = CH_ROWS // NCORES   # 64 rows per core per chunk
EP = E + 1        # vW columns + folded ones column (softmax denominator)
NA = 257          # first-half free dim of the split final matmul
NBC = EP - NA     # 256; its last column holds the denominator

_cache = {}
last_result = None


def _build_nc():
    from concourse import bacc, tile
    from concourse.bass import mybir

    bf16 = mybir.dt.bfloat16
    f16 = mybir.dt.float16
    f32 = mybir.dt.float32

    nc = bacc.Bacc(
        "TRN2", target_bir_lowering=False, debug=False, num_devices=NCORES
    )

    xT = nc.dram_tensor("xT", [E, T], bf16, kind="ExternalInput")
    yT = nc.dram_tensor("yT", [E, T], bf16, kind="ExternalInput")
    gw = nc.dram_tensor("gw", [E, E], bf16, kind="ExternalInput")   # Wk Wq^T
    w2 = nc.dram_tensor("w2", [E, E], bf16, kind="ExternalInput")   # Wv Wu
    # fp16 reduction payload: the partials are ~N(0, 0.1^2) so fp16 keeps
    # ~3 more mantissa bits than bf16 and halves the collective bytes.
    out_ext = nc.dram_tensor("out", [NCH, RS_ROWS, E], f16, kind="ExternalOutput")

    with tile.TileContext(nc) as tc:
        with (
            tc.tile_pool(name="persist", bufs=1) as persist,
            tc.tile_pool(name="work", bufs=4) as work,
            tc.tile_pool(name="expp", bufs=32) as expp,
            tc.tile_pool(name="psum", bufs=2, space="PSUM") as psum_pool,
            tc.tile_pool(name="dram", bufs=1, space="DRAM") as dram,
        ):
            def alloc_rows(prefix, n):
                return [
                    persist.tile(
                        [128, n], bf16, tag=f"{prefix}{j}", name=f"{prefix}{j}"
                    )
                    for j in range(NE)
                ]

            xT_sb = alloc_rows("xTs", T)
            yT_sb = alloc_rows("yTs", T)
            gw_sb = alloc_rows("gws", E)
            w2_sb = alloc_rows("w2s", E)

            # DMA order = need order: gw, xT col-chunk 0, w2, rest of xT,
            # then yT per column block.  Column-chunked so the first
            # projection matmuls can start after ~1MB has landed.
            # first-needed tiles split in half so they spread across more
            # DMA queues and the first matmul can start sooner
            for j in range(NE):
                for h in range(2):
                    nc.sync.dma_start(
                        gw_sb[j][:, h * 256 : (h + 1) * 256],
                        gw[j * 128 : (j + 1) * 128, h * 256 : (h + 1) * 256],
                    )
                for h in range(2):
                    nc.sync.dma_start(
                        xT_sb[j][:, h * 256 : (h + 1) * 256],
                        xT[j * 128 : (j + 1) * 128, h * 256 : (h + 1) * 256],
                    )
            for tb in range(1, NB):
                for j in range(NE):
                    nc.sync.dma_start(
                        xT_sb[j][:, tb * TB : (tb + 1) * TB],
                        xT[j * 128 : (j + 1) * 128, tb * TB : (tb + 1) * TB],
                    )
            for j in range(NE):
                nc.sync.dma_start(w2_sb[j][:], w2[j * 128 : (j + 1) * 128, :])
            for tb in range(NB):
                for j in range(NE):
                    nc.sync.dma_start(
                        yT_sb[j][:, tb * TB : (tb + 1) * TB],
                        yT[j * 128 : (j + 1) * 128, tb * TB : (tb + 1) * TB],
                    )

            gT_sb = alloc_rows("gTs", T)
            # vW plus a folded ones column: col E is 1.0, so the final
            # matmul's second half also produces the softmax denominator.
            vW_sb = [
                persist.tile([128, EP], bf16, tag=f"vWs{t}", name=f"vWs{t}")
                for t in range(NT)
            ]

            zbias = persist.tile([128, 1], f32, tag="zbias", name="zbias")
            nc.vector.memset(zbias[:], 0.0)

            # gT[m][:, tk] = sum_j G[j][:, m-slice].T @ xT[j][:, tk-block]
            for m in range(NE):
                for tb in range(NB):
                    ps = psum_pool.tile(
                        [128, TB], f32, tag="mm", bufs=4, name="ps_g"
                    )
                    for j in range(NE):
                        nc.tensor.matmul(
                            ps[:],
                            gw_sb[j][:, m * 128 : (m + 1) * 128],
                            xT_sb[j][:, tb * TB : (tb + 1) * TB],
                            start=(j == 0),
                            stop=(j == NE - 1),
                        )
                    nc.vector.tensor_copy(gT_sb[m][:, tb * TB : (tb + 1) * TB], ps[:])

            # vW[t, :] = x @ W2   (natural [t, e] layout), ones in col E
            for t in range(NT):
                ps = psum_pool.tile([128, E], f32, tag="mm", bufs=4, name="ps_vw")
                for j in range(NE):
                    nc.tensor.matmul(
                        ps[:],
                        xT_sb[j][:, t * 128 : (t + 1) * 128],
                        w2_sb[j][:],
                        start=(j == 0),
                        stop=(j == NE - 1),
                    )
                nc.vector.memset(vW_sb[t][:, E:EP], 1.0)
                nc.vector.tensor_copy(vW_sb[t][:, 0:E], ps[:])

            SCALE = float(E) ** -0.5
            parts = [
                dram.tile([CH_ROWS, E], f16, tag=f"part{c}", name=f"part{c}")
                for c in range(NCH)
            ]
            rs_outs = [
                dram.tile([RS_ROWS, E], f16, tag=f"rso{c}", name=f"rso{c}")
                for c in range(NCH)
            ]

            for b in range(NB):
                # scoresT[tk, tq-block b] then exp
                exp_tiles = []
                for tk in range(NT):
                    ps = psum_pool.tile(
                        [128, TB], f32, tag="mm", bufs=4, name="ps_sc"
                    )
                    for m in range(NE):
                        nc.tensor.matmul(
                            ps[:],
                            gT_sb[m][:, tk * 128 : (tk + 1) * 128],
                            yT_sb[m][:, b * TB : (b + 1) * TB],
                            start=(m == 0),
                            stop=(m == NE - 1),
                        )
                    et = expp.tile([128, TB], bf16, tag="expT", bufs=32, name="et")
                    nc.scalar.activation(
                        et[:],
                        ps[:],
                        mybir.ActivationFunctionType.Exp,
                        bias=zbias[:],
                        scale=SCALE,
                    )
                    exp_tiles.append(et)

                # out rows for this block: accumulate over tk, then normalize.
                # The EP=513 free dim is split 257+256 across two PSUM banks;
                # the last column of pb is the softmax denominator.
                for qi in range(TB // 128):
                    pa = psum_pool.tile([128, NA], f32, tag="acca", bufs=2, name="pa")
                    pb = psum_pool.tile([128, NBC], f32, tag="accb", bufs=2, name="pb")
                    for tk in range(NT):
                        lhs = exp_tiles[tk][:, qi * 128 : (qi + 1) * 128]
                        nc.tensor.matmul(
                            pa[:],
                            lhs,
                            vW_sb[tk][:, 0:NA],
                            start=(tk == 0),
                            stop=(tk == NT - 1),
                        )
                        nc.tensor.matmul(
                            pb[:],
                            lhs,
                            vW_sb[tk][:, NA:EP],
                            start=(tk == 0),
                            stop=(tk == NT - 1),
                        )
                    rec = work.tile([128, 1], f32, tag="rec", bufs=4, name="rec")
                    nc.vector.reciprocal(rec[:], pb[:, NBC - 1 : NBC])
                    ot = work.tile([128, E], f16, tag="ot", bufs=4, name="ot")
                    nc.vector.tensor_scalar_mul(ot[:, 0:NA], pa[:], rec[:])
                    nc.vector.tensor_scalar_mul(
                        ot[:, NA:E], pb[:, 0 : NBC - 1], rec[:]
                    )
                    nc.sync.dma_start(
                        parts[b][qi * 128 : (qi + 1) * 128, :], ot[:]
                    )
                nc.gpsimd.collective_compute(
                    "ReduceScatter",
                    mybir.AluOpType.add,
                    replica_groups=[list(range(NCORES))],
                    ins=[parts[b][:]],
                    outs=[rs_outs[b][:]],
                )
                nc.sync.dma_start(out_ext[b], rs_outs[b][:])

    nc.compile()
    return nc


def kernel(x, y, Wk, Wq, Wv, Wu, bu):
    global last_result
    from concourse.bass_utils import run_bass_kernel_spmd

    if "nc" not in _cache:
        _cache["nc"] = _build_nc()
    nc = _cache["nc"]

    bf = ml_dtypes.bfloat16
    x = np.asarray(x, np.float32)
    y = np.asarray(y, np.float32)
    Wk = np.asarray(Wk, np.float32)
    Wq = np.asarray(Wq, np.float32)
    Wv = np.asarray(Wv, np.float32)
    Wu = np.asarray(Wu, np.float32)

    xT = np.ascontiguousarray(x.T).astype(bf)
    yT = np.ascontiguousarray(y.T).astype(bf)

    in_maps = []
    for i in range(NCORES):
        sl = slice(i * E, (i + 1) * E)
        G = Wk[:, sl] @ Wq[:, sl].T        # [e, e] fp32 on host
        W2 = Wv[:, sl] @ Wu[sl, :]         # [e, e] fp32 on host
        in_maps.append(
            {
                "xT": xT,
                "yT": yT,
                "gw": G.astype(bf),
                "w2": W2.astype(bf),
            }
        )

    trace = os.environ.get("KERNEL_TRACE", "0") == "1"
    res = run_bass_kernel_spmd(
        nc, in_maps, core_ids=list(range(NCORES)), trace=trace
    )
    last_result = res

    out_full = np.empty((T, E), np.float32)
    for i in range(NCORES):
        o = np.asarray(res.results[i]["out"]).astype(np.float32)  # [NCH, RS_ROWS, E]
        for c in range(NCH):
            r0 = c * CH_ROWS + i * RS_ROWS
            out_full[r0 : r0 + RS_ROWS] = o[c]
    out_full = out_full + np.asarray(bu, np.float32)[None, :]
    return out_full[None]
